# revision 1
# baseline (speedup 1.0000x reference)
"""Differential attention (B=2, S=2048, D=2048, H=16) on 8 Trainium2 cores.

Sharding: core c -> batch b=c//4, head group hg=c%4 (4 heads each).
Fully fused single-pass kernel, all matmuls in bf16:
  per 512-token chunk: qkv projection -> RoPE (bf16 elementwise on DVE/Pool)
  -> scatter into per-head SBUF K/V/Q (no DRAM roundtrip); then per
  256-token attention chunk: scores [k,q] -> exp (one ACT op spans both
  maps) -> flipped AV (stationary = exp-block, moving = v with appended
  ones / (1/lambda) columns) accumulating U[q, vd+denom] in PSUM, so the
  softmax denominators ride free in columns 128/129 -> LN stats via DVE
  accum_out -> normalize (ACT per-partition scale/bias) -> DMA-transpose
  -> partial @ W_o interleaved one chunk behind as PE filler.
ln_gamma * (1-LBDA_INIT) is folded into W_o rows host-side; the ln_beta
rank-1 term is added on the host after the gather.
"""
import sys

sys.path.insert(0, "/opt/trn_rl_repo")

import numpy as np
import ml_dtypes

B, S, D = 2, 2048, 2048
H = 16
HD = D // (2 * H)          # 64 per-map head dim
DH = 2 * HD                # 128 per-head dim
HPC = H // 4               # 4 heads per core
NCORES = 8
SCALE = HD ** -0.5         # 0.125
NEG = -8.0e9               # mask add value pre-scale (-1e9 / SCALE)
OUT_MULT = 1.0 - 0.8       # (1 - LBDA_INIT)

NCH = S // 512             # 4 projection chunks
NQC = S // 256             # 8 attention chunks
KT = S // 128              # 16 k tiles
KO = D // 128              # 16 contraction chunks

# module-level knobs / results for test.py
TRACE = False
TRACE_DIR = None
LAST_RESULTS = None
LAST_EXEC_NS = None

_PROGRAM_CACHE = {}

bf16_np = ml_dtypes.bfloat16


def build_program(s=S):
    """Build the per-core Bass program (SPMD: same program, 8 cores)."""
    import os
    BISECT = os.environ.get("KBISECT", "full")
    import concourse.bass as bass
    import concourse.tile as tile
    from concourse import bacc, mybir
    from concourse.bass import ts, ds

    f32 = mybir.dt.float32
    bf16 = mybir.dt.bfloat16
    AF = mybir.ActivationFunctionType
    OP = mybir.AluOpType

    nc = bacc.Bacc()
    xT = nc.declare_dram_parameter("xT", [D, s], bf16, isOutput=False)
    wqk = nc.declare_dram_parameter("wqk", [D, 8 * 128], bf16, isOutput=False)
    wv = nc.declare_dram_parameter("wv", [D, HPC * DH], bf16, isOutput=False)
    wo = nc.declare_dram_parameter("wo", [HPC * DH, D], bf16, isOutput=False)
    cs = nc.declare_dram_parameter("cs", [128, s], bf16, isOutput=False)
    sn = nc.declare_dram_parameter("sn", [128, s], bf16, isOutput=False)
    dg = nc.declare_dram_parameter("dg", [128, 256], f32, isOutput=False)
    lamv = nc.declare_dram_parameter("lamv", [128, 2], bf16, isOutput=False)
    out = nc.declare_dram_parameter("out", [s, D], bf16, isOutput=True)

    with tile.TileContext(nc) as tc:
        with tc.tile_pool(name="pw", bufs=1) as pw, \
             tc.tile_pool(name="px", bufs=2) as px, \
             tc.tile_pool(name="pq", bufs=2) as pq, \
             tc.tile_pool(name="pr", bufs=2) as pr, \
             tc.tile_pool(name="pe", bufs=4) as pe_pool, \
             tc.tile_pool(name="pep", bufs=4) as pep, \
             tc.tile_pool(name="pat", bufs=8) as pat, \
             tc.tile_pool(name="pst", bufs=2) as pst, \
             tc.tile_pool(name="pao", bufs=2) as pao, \
             tc.tile_pool(name="pos", bufs=3) as pos, \
             tc.tile_pool(name="pps", bufs=1, space="PSUM") as pps:

            # ---------------- persistent SBUF ---------------------------
            # interleave chunk-0 x loads with wqk so the first projection
            # matmuls can start within a few microseconds
            qs = [nc.sync, nc.gpsimd, nc.scalar]
            wqk_sb = pw.tile([128, KO, 8 * 128], bf16)
            xt0 = px.tile([128, KO, 512], bf16, name="xt0", tag="xt")
            for ko in range(KO):
                qs[ko % 3].dma_start(xt0[:, ko, :], xT[ds(ko * 128, 128),
                                                       ts(0, 512)])
                qs[(ko + 1) % 3].dma_start(wqk_sb[:, ko, :],
                                           wqk[ds(ko * 128, 128), :])
            cs_sb = pw.tile([128, s], bf16)
            nc.scalar.dma_start(cs_sb[:], cs[:])
            sn_sb = pw.tile([128, s], bf16)
            nc.sync.dma_start(sn_sb[:], sn[:])
            wv_sb = pw.tile([128, KO, HPC * DH], bf16)
            for ko in range(KO):
                qs[ko % 3].dma_start(wv_sb[:, ko, :], wv[ds(ko * 128, 128), :])
            dg_sb = pw.tile([128, 256], f32)
            nc.sync.dma_start(dg_sb[:], dg[:])
            dgv = dg_sb[:].rearrange("p (m c) -> p m c", m=2)
            lam_sb = pw.tile([128, 2], bf16)
            nc.gpsimd.dma_start(lam_sb[:], lamv[:])

            k_sb = pw.tile([64, 2, HPC, s], bf16)
            v_sb = pw.tile([128, KT, HPC, DH + 2], bf16)
            # denominator columns: col 128 = 1.0, col 129 = 1/lambda
            nc.gpsimd.dma_start(
                out=v_sb[:, :, :, DH:DH + 2].rearrange("p a b c -> p (a b) c"),
                in_=lam_sb[:].unsqueeze(1).to_broadcast([128, KT * HPC, 2]))
            wo_sb = pw.tile([128, HPC, D], bf16)
            for hh in range(HPC):
                qs[hh % 3].dma_start(wo_sb[:, hh, :], wo[ds(hh * DH, DH), :])

            qtiles = {}

            # ---------------- projection + rope for one 512-chunk -------
            def emit_proj(ncI):
                if ncI == 0:
                    xt = xt0
                else:
                    xt = px.tile([128, KO, 512], bf16, name=f"xt{ncI}", tag="xt")
                    for ko in range(KO):
                        eng = nc.sync if ko % 2 == 0 else nc.gpsimd
                        eng.dma_start(xt[:, ko, :],
                                      xT[ds(ko * 128, 128), ts(ncI, 512)])
                q_sb = pq.tile([64, 2, HPC, 512], bf16, name=f"q{ncI}", tag="q")
                qtiles[ncI] = q_sb
                csc = cs_sb[:, ts(ncI, 512)]
                snc = sn_sb[:, ts(ncI, 512)]
                for pair in (2, 3, 0, 1):       # k maps first, v between, q last
                    if pair == 0:
                        emit_v(ncI, xt)
                    be, bo = 2 * pair, 2 * pair + 1
                    pe_ = pps.tile([128, 512], f32, tag="proj", bufs=2,
                                   name=f"pe{ncI}{pair}")
                    po_ = pps.tile([128, 512], f32, tag="proj", bufs=2,
                                   name=f"po{ncI}{pair}")
                    for ko in range(KO):
                        nc.tensor.matmul(pe_[:], wqk_sb[:, ko, ts(be, 128)],
                                         xt[:, ko, :], start=(ko == 0),
                                         stop=(ko == KO - 1))
                    for ko in range(KO):
                        nc.tensor.matmul(po_[:], wqk_sb[:, ko, ts(bo, 128)],
                                         xt[:, ko, :], start=(ko == 0),
                                         stop=(ko == KO - 1))
                    peb = pr.tile([128, 512], bf16, tag="peb")
                    pob = pr.tile([128, 512], bf16, tag="pob")
                    nc.scalar.activation(peb[:], pe_[:], AF.Copy)
                    nc.vector.tensor_copy(out=pob[:], in_=po_[:])
                    t1 = pr.tile([128, 512], bf16, tag="t1")
                    t2 = pr.tile([128, 512], bf16, tag="t2")
                    oe = pr.tile([128, 512], bf16, tag="oe")
                    nc.vector.tensor_tensor(t1[:], peb[:], csc, OP.mult)
                    nc.vector.tensor_tensor(t2[:], pob[:], snc, OP.mult)
                    nc.vector.tensor_tensor(oe[:], t1[:], t2[:], OP.subtract)
                    t3 = pr.tile([128, 512], bf16, tag="t3")
                    t4 = pr.tile([128, 512], bf16, tag="t4")
                    oo = pr.tile([128, 512], bf16, tag="oo")
                    nc.gpsimd.tensor_tensor(t3[:], peb[:], snc, OP.mult)
                    nc.gpsimd.tensor_tensor(t4[:], pob[:], csc, OP.mult)
                    nc.gpsimd.tensor_tensor(oo[:], t3[:], t4[:], OP.add)
                    # scatter into per-head layout [m1e m1o m2e m2o]
                    m = pair % 2
                    if pair >= 2:
                        for hh in range(HPC):
                            nc.sync.dma_start(
                                k_sb[ds(0, 32), m, hh, ts(ncI, 512)],
                                oe[ds(hh * 32, 32), :])
                            nc.gpsimd.dma_start(
                                k_sb[ds(32, 32), m, hh, ts(ncI, 512)],
                                oo[ds(hh * 32, 32), :])
                    else:
                        for hh in range(HPC):
                            nc.sync.dma_start(
                                q_sb[ds(0, 32), m, hh, :],
                                oe[ds(hh * 32, 32), :])
                            nc.gpsimd.dma_start(
                                q_sb[ds(32, 32), m, hh, :],
                                oo[ds(hh * 32, 32), :])

            def emit_v(ncI, xt):
                for tsub in range(4):
                    pv = pps.tile([128, HPC * DH], f32, tag="proj", bufs=2,
                                  name=f"pv{ncI}{tsub}")
                    for ko in range(KO):
                        nc.tensor.matmul(pv[:], xt[:, ko, ts(tsub, 128)],
                                         wv_sb[:, ko, :], start=(ko == 0),
                                         stop=(ko == KO - 1))
                    kt = ncI * 4 + tsub
                    nc.any.tensor_copy(
                        out=v_sb[:, kt, :, 0:DH],
                        in_=pv[:].rearrange("p (h d) -> p h d", h=HPC))

            # ---------------- attention for one (head, 256-chunk) -------
            pend = {}   # (h, qc, ki) -> e12, cross-head score warm-start

            def emit_scores(h, qc, ki):
                qoff = 256 * (qc % 2)
                q_sb = qtiles[qc // 2]
                c0 = 128 if ki == 2 * qc + 1 else 0
                s12 = pps.tile([128, 512], f32, tag="sc", bufs=2,
                               name=f"s12_{h}_{qc}_{ki}")
                for m in (0, 1):
                    nc.tensor.matmul(
                        s12[:, ds(256 * m + c0, 256 - c0)],
                        k_sb[:, m, h, ts(ki, 128)],
                        q_sb[:, m, h, ds(qoff + c0, 256 - c0)],
                        start=(m == 0), stop=True, skip_group_check=True)
                s12v = s12[:].rearrange("p (m c) -> p m c", m=2)
                import os as _os
                if _os.environ.get("KBISECT", "full") != "attn_nomask":
                    if ki == 2 * qc:        # diagonal block of subtile 0
                        nc.vector.tensor_tensor(s12v[:, :, 0:128],
                                                s12v[:, :, 0:128], dgv, OP.add)
                    elif ki == 2 * qc + 1:  # diagonal block of subtile 1
                        nc.vector.tensor_tensor(s12v[:, :, 128:256],
                                                s12v[:, :, 128:256], dgv, OP.add)
                e12 = pe_pool.tile([128, 512], bf16, tag="e",
                                   name=f"e12_{h}_{qc}_{ki}")
                e12v = e12[:].rearrange("p (m c) -> p m c", m=2)
                nc.scalar.activation(e12v[:, :, c0:256], s12v[:, :, c0:256],
                                     AF.Exp, scale=SCALE)
                return e12

            def emit_attn(h, qc, sA, sS, attp):
                klim = 2 * qc + 2

                U = [[None, None], [None, None]]
                for j in (0, 1):
                    for m in (0, 1):
                        U[j][m] = pps.tile([128, DH + 2], f32, tag=f"u{j}{m}",
                                           name=f"U{h}{qc}{j}{m}")

                if (h, qc, 0) not in pend:
                    pend[(h, qc, 0)] = emit_scores(h, qc, 0)
                for ki in range(klim):
                    if ki + 1 < klim and (h, qc, ki + 1) not in pend:
                        pend[(h, qc, ki + 1)] = emit_scores(h, qc, ki + 1)
                    e12 = pend.pop((h, qc, ki))
                    for j in (0, 1):
                        lim = 2 * qc + j
                        if ki > lim:
                            continue
                        for m in (0, 1):
                            nc.tensor.matmul(
                                U[j][m][:, 0:DH + 1 + m],
                                e12[:, ds(256 * m + 128 * j, 128)],
                                v_sb[:, ki, h, 0:DH + 1 + m],
                                start=(ki == 0), stop=(ki == lim))
                # warm-start the next (head, chunk)'s first score tiles so
                # its AV never waits on a cold exp
                if h + 1 < HPC:
                    nh, nqc = h + 1, qc
                elif qc + 1 < NQC:
                    nh, nqc = 0, qc + 1     # next chunk's proj already emitted
                else:
                    nh = None
                if nh is not None:
                    pend[(nh, nqc, 0)] = emit_scores(nh, nqc, 0)
                    pend[(nh, nqc, 1)] = emit_scores(nh, nqc, 1)

                # epilogue: normalize by denominators, LN partial sums
                import os as _os
                if _os.environ.get("KBISECT", "full") == "attn_noep":
                    return
                for j in (0, 1):
                    col = 2 * h + j
                    r1 = pep.tile([128, 1], f32, tag="r1")
                    r2 = pep.tile([128, 1], f32, tag="r2")
                    nc.vector.reciprocal(out=r1[:], in_=U[j][0][:, DH:DH + 1])
                    nc.vector.reciprocal(out=r2[:], in_=U[j][1][:, DH + 1:DH + 2])
                    t2s = pep.tile([128, 128], f32, tag="t2s")
                    nc.vector.tensor_scalar(out=t2s[:], in0=U[j][1][:, 0:DH],
                                            scalar1=r2[:], scalar2=None,
                                            op0=OP.mult)
                    ap = attp[2 * h + j]
                    nc.vector.scalar_tensor_tensor(
                        out=ap[:], in0=U[j][0][:, 0:DH], scalar=r1[:],
                        in1=t2s[:], op0=OP.mult, op1=OP.subtract,
                        accum_out=sA[:, col:col + 1])
                    sqs = pep.tile([128, 128], bf16, tag="sqs")
                    nc.vector.scalar_tensor_tensor(
                        out=sqs[:], in0=ap[:], scalar=1.0, in1=ap[:],
                        op0=OP.mult, op1=OP.mult,
                        accum_out=sS[:, col:col + 1])

            # ---------------- LN finalize + transpose for one 256-chunk -
            def emit_norm(qc, sA, sS, attp, attnT):
                nmu = pst.tile([128, 8], f32, tag="nmu")
                ex2 = pst.tile([128, 8], f32, tag="ex2")
                nc.vector.tensor_scalar(out=nmu[:], in0=sA[:], scalar1=-1.0 / DH,
                                        scalar2=None, op0=OP.mult)
                nc.vector.tensor_scalar(out=ex2[:], in0=sS[:], scalar1=1.0 / DH,
                                        scalar2=None, op0=OP.mult)
                msq = pst.tile([128, 8], f32, tag="msq")
                nc.vector.tensor_tensor(msq[:], nmu[:], nmu[:], OP.mult)
                var = pst.tile([128, 8], f32, tag="var")
                nc.vector.tensor_tensor(var[:], ex2[:], msq[:], OP.subtract)
                # rsqrt(var + eps) via Quake bit-trick + 2 Newton steps
                # (keeps the whole LN finalize off ACT: no act-table thrash)
                vps = pst.tile([128, 8], f32, tag="vps")
                nc.vector.tensor_scalar(out=vps[:], in0=var[:], scalar1=1e-5,
                                        scalar2=None, op0=OP.add)
                i32 = mybir.dt.int32
                ysh = pst.tile([128, 8], f32, tag="ysh")
                nc.vector.tensor_scalar(out=ysh[:].bitcast(i32),
                                        in0=vps[:].bitcast(i32), scalar1=1,
                                        scalar2=None, op0=OP.arith_shift_right)
                y0 = pst.tile([128, 8], f32, tag="y0")
                nc.vector.tensor_scalar(out=y0[:].bitcast(i32),
                                        in0=ysh[:].bitcast(i32), scalar1=-1,
                                        scalar2=0x5f3759df, op0=OP.mult,
                                        op1=OP.add)
                rstd = y0
                for it in range(2):
                    yy = pst.tile([128, 8], f32, tag=f"yy{it}")
                    nc.vector.tensor_tensor(yy[:], rstd[:], rstd[:], OP.mult)
                    yv = pst.tile([128, 8], f32, tag=f"yv{it}")
                    nc.vector.tensor_tensor(yv[:], yy[:], vps[:], OP.mult)
                    yc = pst.tile([128, 8], f32, tag=f"yc{it}")
                    nc.vector.tensor_scalar(out=yc[:], in0=yv[:], scalar1=-0.5,
                                            scalar2=1.5, op0=OP.mult, op1=OP.add)
                    yn = pst.tile([128, 8], f32, tag=f"yn{it}")
                    nc.vector.tensor_tensor(yn[:], rstd[:], yc[:], OP.mult)
                    rstd = yn
                nbias = pst.tile([128, 8], f32, tag="nbias")
                nc.vector.tensor_tensor(nbias[:], nmu[:], rstd[:], OP.mult)
                for j in (0, 1):
                    for h in range(HPC):
                        col = 2 * h + j
                        attn = pat.tile([128, 128], bf16, tag="attn",
                                        name=f"attn{qc}{h}{j}")
                        nc.any.tensor_scalar(out=attn[:], in0=attp[col][:],
                                             scalar1=rstd[:, col:col + 1],
                                             scalar2=nbias[:, col:col + 1],
                                             op0=OP.mult, op1=OP.add)
                        nc.sync.dma_start_transpose(
                            attnT[:, h, ds(128 * j, 128)], attn[:])
                    if qc == NQC - 1:
                        # last chunk: start W_o on this token-half right away
                        emit_wo_piece(qc, attnT, 2 * j)
                        emit_wo_piece(qc, attnT, 2 * j + 1)

            # ---------------- W_o partial for one 256-chunk -------------
            def emit_wo_piece(qc, attnT, piece):
                # piece in 0..3, two (qi, nj) groups each
                for g in (2 * piece, 2 * piece + 1):
                    qi, nj = g // 4, g % 4
                    po = pps.tile([128, 512], f32, tag="proj", bufs=2,
                                  name=f"wo{qc}{qi}{nj}")
                    for hh in range(HPC):
                        nc.tensor.matmul(po[:],
                                         attnT[:, hh, ds(128 * qi, 128)],
                                         wo_sb[:, hh, ts(nj, 512)],
                                         start=(hh == 0),
                                         stop=(hh == HPC - 1))
                    ost = pos.tile([128, 512], bf16, tag="ost")
                    nc.any.tensor_copy(out=ost[:], in_=po[:])
                    eng = nc.sync if nj % 2 == 0 else nc.gpsimd
                    eng.dma_start(out[ds(qc * 256 + qi * 128, 128),
                                      ts(nj, 512)], ost[:])

            # ---------------- main fused loop ---------------------------
            attnTs = {}
            emit_proj(0)
            for ncI in range(NCH):
                for half in (0, 1):
                    qc = 2 * ncI + half
                    if BISECT == "attn_half" and qc >= 4:
                        break
                    if BISECT in ("proj",):
                        if half == 0 and ncI + 1 < NCH:
                            emit_proj(ncI + 1)
                        continue
                    sA = pst.tile([128, 8], f32, tag="sA", name=f"sA{qc}")
                    sS = pst.tile([128, 8], f32, tag="sS", name=f"sS{qc}")
                    attp = [pat.tile([128, 128], f32, tag=f"attp{i}",
                                     name=f"attp{qc}_{i}", bufs=1)
                            for i in range(8)]
                    pieces = [[0], [1], [2], [3]] if qc <= 4 else \
                        [[], [0], [1, 2], [3]]
                    for h in range(HPC):
                        if BISECT == "attn_one" and (qc != 0 or h != 0):
                            continue
                        emit_attn(h, qc, sA, sS, attp)
                        # W_o of the previous chunk, sprinkled between heads
                        # so PE has ready filler at every head boundary
                        if qc >= 1 and BISECT == "full":
                            for pc in pieces[h]:
                                emit_wo_piece(qc - 1, attnTs[qc - 1], pc)
                    if qc >= 1 and BISECT == "full":
                        attnTs.pop(qc - 1)
                    if BISECT in ("norm", "full"):
                        attnT = pao.tile([128, HPC, 256], bf16,
                                         name=f"attnT{qc}", tag="attnT")
                        attnTs[qc] = attnT
                        emit_norm(qc, sA, sS, attp, attnT)
                    # next chunk's projection between the two halves, so the
                    # scheduler has dense PE work during attention phases
                    if half == 0 and ncI + 1 < NCH:
                        emit_proj(ncI + 1)

            if BISECT != "full":
                dwr = pos.tile([128, 512], bf16, tag="ost")
                nc.gpsimd.memset(dwr[:], 0.0)
                nc.sync.dma_start(out[ds(0, 128), ts(0, 512)], dwr[:])

    nc.finalize()
    return nc


def get_program(s=S):
    if s not in _PROGRAM_CACHE:
        _PROGRAM_CACHE[s] = build_program(s)
    return _PROGRAM_CACHE[s]


def make_core_inputs(x, cos, sin, W_qkv, W_o, ln_gamma, lbda, core, s=S):
    """Host-side shard prep for one core."""
    b, hg = core // 4, core % 4
    heads = list(range(hg * HPC, (hg + 1) * HPC))

    def qk_block_cols(base, dstart):
        # even/odd pair columns for one 32-wide block across the 4 heads
        return [base + hh * DH + dstart + 2 * p for hh in heads for p in range(32)]

    cols = []
    for base in (0, D):                       # q section, k section
        for dstart in (0, 1, HD, HD + 1):     # m1-even, m1-odd, m2-even, m2-odd
            cols += qk_block_cols(base, dstart)
    wqk = np.ascontiguousarray(W_qkv[:, cols]).astype(bf16_np)
    vcols = [2 * D + hh * DH + dd for hh in heads for dd in range(DH)]
    wv = np.ascontiguousarray(W_qkv[:, vcols]).astype(bf16_np)
    worows = [hh * DH + dd for hh in heads for dd in range(DH)]
    gamma_scale = np.concatenate([ln_gamma[hh] * OUT_MULT for hh in heads])
    wo = np.ascontiguousarray(W_o[worows, :] * gamma_scale[:, None]).astype(bf16_np)

    xT = np.ascontiguousarray(x[b].T).astype(bf16_np)
    cst = np.ascontiguousarray(np.tile(cos.T, (HPC, 1))).astype(bf16_np)
    snt = np.ascontiguousarray(np.tile(sin.T, (HPC, 1))).astype(bf16_np)

    diag = np.where(np.triu(np.ones((128, 128), dtype=bool)), 0.0, NEG)
    dg2 = np.concatenate([diag, diag], axis=1).astype(np.float32)

    lam2 = np.zeros((128, 2), dtype=np.float32)
    lam2[:, 0] = 1.0
    lam2[:, 1] = 1.0 / max(float(lbda), 1e-6)

    return {
        "xT": xT, "wqk": wqk, "wv": wv, "wo": wo, "cs": cst, "sn": snt,
        "dg": dg2, "lamv": lam2.astype(bf16_np),
    }


def _mask_is_causal(mask, s=S):
    m = np.asarray(mask).reshape(s, s)
    tril = np.tril(np.ones((s, s), dtype=bool))
    if not np.array_equal(m == 0.0, tril):
        return False
    off = m[~tril]
    return off.size == 0 or (np.all(off <= -1.0e8) and np.all(np.isfinite(off)))


def _numpy_reference(x, mask, cos, sin, W_qkv, W_o, ln_gamma, ln_beta, lbda):
    """Exact-math fallback (used only if the mask is not the causal pattern)."""
    b, s, d = x.shape
    qkv = x @ W_qkv
    q, k, v = np.split(qkv, 3, axis=-1)
    q = q.reshape(b, s, H, DH).transpose(0, 2, 1, 3)
    k = k.reshape(b, s, H, DH).transpose(0, 2, 1, 3)
    v = v.reshape(b, s, H, DH).transpose(0, 2, 1, 3)

    def rope(t):
        tr = t.reshape(b, H, s, HD // 2, 2)
        x1, x2 = tr[..., 0], tr[..., 1]
        c = cos[None, None]
        sn_ = sin[None, None]
        o1 = x1 * c - x2 * sn_
        o2 = x1 * sn_ + x2 * c
        return np.stack([o1, o2], axis=-1).reshape(b, H, s, HD)

    q1, q2 = q[..., :HD], q[..., HD:]
    k1, k2 = k[..., :HD], k[..., HD:]
    q1, k1 = rope(q1), rope(k1)
    q2, k2 = rope(q2), rope(k2)

    def softm(z):
        z = z - z.max(-1, keepdims=True)
        e = np.exp(z)
        return e / e.sum(-1, keepdims=True)

    m = np.asarray(mask).reshape(1, 1, s, s)
    a1 = softm(np.einsum("bhqd,bhkd->bhqk", q1, k1) * SCALE + m)
    a2 = softm(np.einsum("bhqd,bhkd->bhqk", q2, k2) * SCALE + m)
    a = a1 - float(lbda) * a2
    o = np.einsum("bhqk,bhkd->bhqd", a, v)
    mu = o.mean(-1, keepdims=True)
    var = o.var(-1, keepdims=True)
    o = (o - mu) / np.sqrt(var + 1e-5)
    o = o * ln_gamma[None, :, None, :] + ln_beta[None, :, None, :]
    o = o * OUT_MULT
    o = o.transpose(0, 2, 1, 3).reshape(b, s, d)
    return (o @ W_o).astype(np.float32)


def kernel(x, mask, cos, sin, W_qkv, W_o, ln_gamma, ln_beta, lbda):
    global LAST_RESULTS, LAST_EXEC_NS
    x = np.asarray(x, dtype=np.float32)
    cos = np.asarray(cos, dtype=np.float32)
    sin = np.asarray(sin, dtype=np.float32)
    W_qkv = np.asarray(W_qkv, dtype=np.float32)
    W_o = np.asarray(W_o, dtype=np.float32)
    ln_gamma = np.asarray(ln_gamma, dtype=np.float32)
    ln_beta = np.asarray(ln_beta, dtype=np.float32)
    lbda_f = float(np.asarray(lbda))

    if not _mask_is_causal(mask):
        return _numpy_reference(x, mask, cos, sin, W_qkv, W_o,
                                ln_gamma, ln_beta, lbda_f)

    from concourse.bass_utils import run_bass_kernel_spmd

    nc = get_program(S)
    in_maps = [
        make_core_inputs(x, cos, sin, W_qkv, W_o, ln_gamma, lbda_f, c)
        for c in range(NCORES)
    ]
    kwargs = {"trace": TRACE}
    if TRACE and TRACE_DIR:
        kwargs["tmpdir"] = TRACE_DIR
    res = run_bass_kernel_spmd(nc, in_maps, core_ids=list(range(NCORES)),
                               **kwargs)
    LAST_RESULTS = res
    LAST_EXEC_NS = getattr(res, "exec_time_ns", None)

    outf = np.zeros((B, S, D), dtype=np.float32)
    for c in range(NCORES):
        outf[c // 4] += res.results[c]["out"].astype(np.float32)
    # ln_beta rank-1 term: (beta * OUT_MULT) @ W_o added to every token
    beta_term = (ln_beta.reshape(-1) * OUT_MULT) @ W_o
    outf += beta_term[None, None, :]
    return outf



# revision 42
# speedup vs baseline: 1.1148x; 1.1148x over previous
"""Differential attention (B=2, S=2048, D=2048, H=16) on 8 Trainium2 cores.

Sharding: core c -> batch b=c//4, head group hg=c%4 (4 heads each).
Fully fused single-pass kernel, all matmuls in bf16:
  per 512-token chunk: qkv projection -> RoPE (elementwise on Pool, PSUM
  copies on DVE) -> scatter into per-head SBUF K/V/Q (no DRAM roundtrip);
  then per 256-token attention chunk: scores for two k-tiles land in one
  2-bank PSUM tile -> a single ACT exp spans both tiles and both maps ->
  flipped AV (stationary = exp-block, moving = v with appended ones and
  1/lambda columns) accumulating U[q, vd+denom] in PSUM (both maps packed
  into one bank per q-half) -> LN stats via DVE accum_out -> per-head
  normalize + DMA-transpose -> partial @ W_o -> bf16 partials to DRAM.

Scheduling: the attention inner loop is the pacing master.  ACT (exp)
needs ~550ns per (h,qc,ki) unit while the unit's own matmuls only take
~430ns, so a filler queue of single-matmul closures (W_o groups lagging
one chunk, the next chunk's projection pieces with per-piece due-dates)
is pumped inside the ki loop to keep PE busy while exp catches up.  ACT
runs exps only in steady state; q projection pieces are forced early
enough to cover the scatter-DMA latency, k/v pieces are deferred into
their consuming chunk as late filler.  The last chunk runs per-head
norm with held-back W_o groups covering the final transposes, the last
head's transpose runs on the PE itself, and the last W_o groups defer
their h3 matmul until its transpose lands.
ln_gamma * (1-LBDA_INIT) is folded into W_o rows host-side; the ln_beta
rank-1 term is added on the host after the gather.
"""
import sys

sys.path.insert(0, "/opt/trn_rl_repo")

import numpy as np
import ml_dtypes

B, S, D = 2, 2048, 2048
H = 16
HD = D // (2 * H)          # 64 per-map head dim
DH = 2 * HD                # 128 per-head dim
HPC = H // 4               # 4 heads per core
NCORES = 8
SCALE = HD ** -0.5         # 0.125
NEG = -8.0e9               # mask add value pre-scale (-1e9 / SCALE)
OUT_MULT = 1.0 - 0.8       # (1 - LBDA_INIT)

NCH = S // 512             # 4 projection chunks
NQC = S // 256             # 8 attention chunks
KT = S // 128              # 16 k tiles
KO = D // 128              # 16 contraction chunks

# module-level knobs / results for test.py
TRACE = False
TRACE_DIR = None
LAST_RESULTS = None
LAST_EXEC_NS = None

_PROGRAM_CACHE = {}

bf16_np = ml_dtypes.bfloat16


def build_program(s=S):
    """Build the per-core Bass program (SPMD: same program, 8 cores)."""
    import os
    import collections
    import concourse.bass as bass
    import concourse.tile as tile
    from concourse import bacc, mybir
    from concourse.bass import ts, ds

    DEPTH = int(os.environ.get("KDEPTH", "2"))
    FILL = float(os.environ.get("KFILL", "520"))

    f32 = mybir.dt.float32
    bf16 = mybir.dt.bfloat16
    AF = mybir.ActivationFunctionType
    OP = mybir.AluOpType

    nc = bacc.Bacc()
    xT = nc.declare_dram_parameter("xT", [D, s], bf16, isOutput=False)
    wqk = nc.declare_dram_parameter("wqk", [D, 8 * 128], bf16, isOutput=False)
    wv = nc.declare_dram_parameter("wv", [D, HPC * DH], bf16, isOutput=False)
    wo = nc.declare_dram_parameter("wo", [HPC * DH, D], bf16, isOutput=False)
    cs = nc.declare_dram_parameter("cs", [128, s], bf16, isOutput=False)
    sn = nc.declare_dram_parameter("sn", [128, s], bf16, isOutput=False)
    dg = nc.declare_dram_parameter("dg", [128, 256], f32, isOutput=False)
    lamv = nc.declare_dram_parameter("lamv", [128, 2], bf16, isOutput=False)
    ident = nc.declare_dram_parameter("ident", [128, 128], bf16, isOutput=False)
    out = nc.declare_dram_parameter("out", [s, D], bf16, isOutput=True)

    with tile.TileContext(nc) as tc:
        with tc.tile_pool(name="pw", bufs=1) as pw, \
             tc.tile_pool(name="px", bufs=2) as px, \
             tc.tile_pool(name="pq", bufs=2) as pq, \
             tc.tile_pool(name="pr", bufs=2) as pr, \
             tc.tile_pool(name="pe", bufs=4) as pe_pool, \
             tc.tile_pool(name="pep", bufs=2) as pep, \
             tc.tile_pool(name="pat", bufs=8) as pat, \
             tc.tile_pool(name="pst", bufs=2) as pst, \
             tc.tile_pool(name="pao", bufs=3) as pao, \
             tc.tile_pool(name="pos", bufs=2) as pos, \
             tc.tile_pool(name="pps", bufs=1, space="PSUM") as pps:

            # ---------------- persistent SBUF ---------------------------
            # interleave chunk-0 x loads with wqk so the first projection
            # matmuls can start within a few microseconds
            qs = [nc.sync, nc.gpsimd, nc.scalar]
            wqk_sb = pw.tile([128, KO, 8 * 128], bf16)
            xt0 = px.tile([128, KO, 512], bf16, name="xt0", tag="xt")
            # the very first PE instruction is the Ldweights of wqk ko=0
            # cols 512:640 (pair 2): give that slice its own small DMA at
            # the head of the sync HWDGE queue
            nc.sync.dma_start(wqk_sb[:, 0, 512:768], wqk[ds(0, 128), 512:768])
            nc.scalar.dma_start(wqk_sb[:, 0, 0:512], wqk[ds(0, 128), 0:512])
            nc.scalar.dma_start(wqk_sb[:, 0, 768:1024],
                                wqk[ds(0, 128), 768:1024])
            for ko in range(KO):
                if ko > 0:
                    qs[ko % 3].dma_start(wqk_sb[:, ko, :],
                                         wqk[ds(ko * 128, 128), :])
                qs[(ko + 1) % 3].dma_start(xt0[:, ko, :], xT[ds(ko * 128, 128),
                                                             ts(0, 512)])
            cs_sb = pw.tile([128, s], bf16)
            nc.scalar.dma_start(cs_sb[:], cs[:])
            sn_sb = pw.tile([128, s], bf16)
            nc.sync.dma_start(sn_sb[:], sn[:])
            wv_sb = pw.tile([128, KO, HPC * DH], bf16)
            for ko in range(KO):
                qs[ko % 3].dma_start(wv_sb[:, ko, :], wv[ds(ko * 128, 128), :])
            dg_sb = pw.tile([128, 256], f32)
            nc.sync.dma_start(dg_sb[:], dg[:])
            dgv = dg_sb[:].rearrange("p (m c) -> p m c", m=2)
            lam_sb = pw.tile([128, 2], bf16)
            nc.gpsimd.dma_start(lam_sb[:], lamv[:])
            id_sb = pw.tile([128, 128], bf16)
            nc.scalar.dma_start(id_sb[:], ident[:])

            k_sb = pw.tile([64, 2, HPC, s], bf16)
            v_sb = pw.tile([128, KT, HPC, DH + 2], bf16)
            # denominator columns: col 128 = 1.0, col 129 = 1/lambda
            nc.gpsimd.dma_start(
                out=v_sb[:, :, :, DH:DH + 2].rearrange("p a b c -> p (a b) c"),
                in_=lam_sb[:].unsqueeze(1).to_broadcast([128, KT * HPC, 2]))
            wo_sb = pw.tile([128, HPC, D], bf16)
            for hh in range(HPC):
                qs[hh % 3].dma_start(wo_sb[:, hh, :], wo[ds(hh * DH, DH), :])

            qtiles = {}
            attnTs = {}

            # ---------------- filler machinery --------------------------
            # closures each emit ~2048 PE cycles; pump() rations them into
            # the attention inner loop so PE never out-runs ACT's exp rate
            # proj entries carry a due-date (qc, ki): q pairs are needed at
            # the warm-start of qc=2*chunk, but k/v parts only at ki=4*chunk
            # inside that qc -- deferring them supplies filler deep into the
            # last chunks where W_o alone can't cover the exp deficit.
            proj_q = collections.deque()   # (cycles, (qc_due, ki_due), closure)
            wo_q = collections.deque()     # (cycles, qc, closure)
            budget = [0.0]

            def pump(extra):
                budget[0] = min(budget[0] + extra, 6000.0)
                while budget[0] > 0 and (proj_q or wo_q):
                    if proj_q:
                        c, _k, fn = proj_q.popleft()
                    else:
                        c, _k, fn = wo_q.popleft()
                    fn()
                    budget[0] -= c

            def drain_proj(due):
                while proj_q and proj_q[0][1] <= due:
                    c, _k, fn = proj_q.popleft()
                    fn()
                    # forced work is still PE filler: debit the pump budget
                    budget[0] = max(budget[0] - c, -16000.0)

            def drain_wo(upto_qc):
                while wo_q and wo_q[0][1] <= upto_qc:
                    c, _k, fn = wo_q.popleft()
                    fn()
                    budget[0] = max(budget[0] - c, -16000.0)

            # ---------------- projection pieces -------------------------
            def rope_pair(ncI, pair, pe_, po_, q_sb):
                """PSUM->bf16 copies + RoPE + scatter (no PE work)."""
                csc = cs_sb[:, ts(ncI, 512)]
                snc = sn_sb[:, ts(ncI, 512)]
                peb = pr.tile([128, 512], bf16, tag="peb")
                pob = pr.tile([128, 512], bf16, tag="pob")
                nc.vector.tensor_copy(out=peb[:], in_=pe_[:])
                nc.vector.tensor_copy(out=pob[:], in_=po_[:])
                t1 = pr.tile([128, 512], bf16, tag="t1")
                t2 = pr.tile([128, 512], bf16, tag="t2")
                oe = pr.tile([128, 512], bf16, tag="oe")
                nc.gpsimd.tensor_tensor(t1[:], peb[:], csc, OP.mult)
                nc.gpsimd.tensor_tensor(t2[:], pob[:], snc, OP.mult)
                nc.gpsimd.tensor_tensor(oe[:], t1[:], t2[:], OP.subtract)
                t3 = pr.tile([128, 512], bf16, tag="t3")
                t4 = pr.tile([128, 512], bf16, tag="t4")
                oo = pr.tile([128, 512], bf16, tag="oo")
                nc.gpsimd.tensor_tensor(t3[:], peb[:], snc, OP.mult)
                nc.gpsimd.tensor_tensor(t4[:], pob[:], csc, OP.mult)
                nc.gpsimd.tensor_tensor(oo[:], t3[:], t4[:], OP.add)
                # scatter into per-head layout
                m = pair % 2
                if pair >= 2:                  # k maps
                    for hh in range(HPC):
                        nc.sync.dma_start(
                            k_sb[ds(0, 32), m, hh, ts(ncI, 512)],
                            oe[ds(hh * 32, 32), :])
                        nc.gpsimd.dma_start(
                            k_sb[ds(32, 32), m, hh, ts(ncI, 512)],
                            oo[ds(hh * 32, 32), :])
                else:                          # q maps
                    for hh in range(HPC):
                        nc.sync.dma_start(
                            q_sb[ds(0, 32), m, hh, :],
                            oe[ds(hh * 32, 32), :])
                        nc.gpsimd.dma_start(
                            q_sb[ds(32, 32), m, hh, :],
                            oo[ds(hh * 32, 32), :])

            def proj_closures(ncI, xt, q_sb):
                """Filler closures for chunk ncI: q pairs first (needed at
                the qc=2*ncI score warm-start right after the drain), then
                v, then k pairs."""
                state = {}

                def mk_qk(pair, which, k0, k1):
                    def fn():
                        key = (pair, which)
                        if key not in state:
                            state[key] = pps.tile(
                                [128, 512], f32, tag="proj", bufs=2,
                                name=f"p{which}{ncI}{pair}")
                        t = state[key]
                        base = 2 * pair + (0 if which == "e" else 1)
                        for ko in range(k0, k1):
                            nc.tensor.matmul(t[:],
                                             wqk_sb[:, ko, ts(base, 128)],
                                             xt[:, ko, :], start=(ko == 0),
                                             stop=(ko == KO - 1))
                        if which == "o" and k1 == KO:
                            rope_pair(ncI, pair, state[(pair, "e")], t, q_sb)
                    return fn

                def mk_v(tsub, k0, k1):
                    def fn():
                        key = ("v", tsub)
                        if key not in state:
                            state[key] = pps.tile(
                                [128, HPC * DH], f32, tag="proj", bufs=2,
                                name=f"pv{ncI}{tsub}")
                        pv = state[key]
                        for ko in range(k0, k1):
                            nc.tensor.matmul(pv[:], xt[:, ko, ts(tsub, 128)],
                                             wv_sb[:, ko, :], start=(ko == 0),
                                             stop=(ko == KO - 1))
                        if k1 == KO:
                            kt = ncI * 4 + tsub
                            nc.vector.tensor_copy(
                                out=v_sb[:, kt, :, 0:DH],
                                in_=pv[:].rearrange("p (h d) -> p h d", h=HPC))
                    return fn

                cls = []
                # q must be scattered (incl. ~3us DMA latency) before the
                # qc=2*chunk warm-start: force it late in the previous qc
                q_due = (2 * ncI - 1, max(0, 4 * ncI - 6))
                for pair in (0, 1):            # q maps first
                    for which in ("e", "o"):
                        for k0 in range(0, KO, 4):
                            cls.append((2048, q_due, mk_qk(pair, which, k0, k0 + 4)))
                # k/v tiles are first touched at ki=4*chunk inside qc=2*chunk;
                # spread dues over early ki of that qc, keeping ~10 units of
                # lead for the scatter-DMA latency before first use
                kv = []
                for pair in (2, 3):            # k maps (needed before late v)
                    for which in ("e", "o"):
                        for k0 in range(0, KO, 4):
                            kv.append(mk_qk(pair, which, k0, k0 + 4))
                for tsub in range(4):          # v
                    for k0 in range(0, KO, 4):
                        kv.append(mk_v(tsub, k0, k0 + 4))
                span = max(4 * ncI - 10, 1)
                for i, fn in enumerate(kv):
                    cls.append((2048, (2 * ncI, (i * span) // len(kv)), fn))
                return cls

            def enqueue_proj(ncI):
                xt = px.tile([128, KO, 512], bf16, name=f"xt{ncI}", tag="xt")
                for ko in range(KO):
                    eng = nc.sync if ko % 2 == 0 else nc.gpsimd
                    eng.dma_start(xt[:, ko, :],
                                  xT[ds(ko * 128, 128), ts(ncI, 512)])
                q_sb = pq.tile([64, 2, HPC, 512], bf16, name=f"q{ncI}", tag="q")
                qtiles[ncI] = q_sb
                proj_q.extend(proj_closures(ncI, xt, q_sb))

            def emit_proj0():
                """Chunk 0, emitted up-front before attention.  Alternates
                PSUM between the proj and sc tags (sc is idle during
                startup) for a 4-deep rotation."""
                xt = xt0
                q_sb = pq.tile([64, 2, HPC, 512], bf16, name="q0", tag="q")
                qtiles[0] = q_sb
                alt = [0]

                def psum_tile(name):
                    tag = "proj" if alt[0] % 2 == 0 else "sc"
                    alt[0] += 1
                    return pps.tile([128, 512], f32, tag=tag,
                                    bufs=(2 if tag == "proj" else 4),
                                    name=name)

                for pair in (2, 3, 0, 1):      # k first, v between, q last
                    if pair == 0:
                        for tsub in range(4):
                            pv = psum_tile(f"pv0{tsub}")
                            for ko in range(KO):
                                nc.tensor.matmul(pv[:, 0:HPC * DH],
                                                 xt[:, ko, ts(tsub, 128)],
                                                 wv_sb[:, ko, :],
                                                 start=(ko == 0),
                                                 stop=(ko == KO - 1))
                            nc.vector.tensor_copy(
                                out=v_sb[:, tsub, :, 0:DH],
                                in_=pv[:, 0:HPC * DH].rearrange(
                                    "p (h d) -> p h d", h=HPC))
                    be, bo = 2 * pair, 2 * pair + 1
                    pe_ = psum_tile(f"pe0{pair}")
                    po_ = psum_tile(f"po0{pair}")
                    for ko in range(KO):
                        nc.tensor.matmul(pe_[:], wqk_sb[:, ko, ts(be, 128)],
                                         xt[:, ko, :], start=(ko == 0),
                                         stop=(ko == KO - 1))
                    for ko in range(KO):
                        nc.tensor.matmul(po_[:], wqk_sb[:, ko, ts(bo, 128)],
                                         xt[:, ko, :], start=(ko == 0),
                                         stop=(ko == KO - 1))
                    rope_pair(0, pair, pe_, po_, q_sb)

            # ---------------- W_o pieces --------------------------------
            # tail staging: e12 tiles are dead by the tail and have the same
            # shape as ost staging, so rotate over both pools (6 slots) to
            # decouple the bunched tail groups from DMA round-trip latency
            stage_i = [0]

            def tail_stage():
                i = stage_i[0]
                stage_i[0] += 1
                if i % 3 == 2:
                    return pos.tile([128, 512], bf16, tag="ost",
                                    name=f"ostp{i}")
                return pe_pool.tile([128, 512], bf16, tag="e",
                                    name=f"ostt{i}")

            def mk_wo_group(qc, g, late=False, held=False):
                attnT = attnTs[qc]
                qi, nj = g // 4, g % 4

                def fn():
                    po = pps.tile([128, 512], f32, tag="sc", bufs=4,
                                  name=f"wo{qc}{qi}{nj}")
                    for hh in range(HPC):
                        nc.tensor.matmul(po[:],
                                         attnT[hh][:, ds(128 * qi, 128)],
                                         wo_sb[:, hh, ts(nj, 512)],
                                         start=(hh == 0),
                                         stop=(hh == HPC - 1))
                    ost = tail_stage() if held else pos.tile([128, 512], bf16,
                                                             tag="ost")
                    if held:
                        # ACT is done with exps by the last chunk's tail;
                        # route held copies there to keep DVE free for the
                        # last head's norm chain
                        nc.scalar.activation(ost[:], po[:], AF.Copy)
                    else:
                        nc.vector.tensor_copy(out=ost[:], in_=po[:])
                    if held:
                        # held groups run while the last chunk's transposes
                        # monopolize the HWDGE: keep their DMAs on SWDGE
                        dmae = nc.gpsimd
                    else:
                        dmae = nc.sync if nj % 2 == 0 else nc.gpsimd
                    dmae.dma_start(out[ds(qc * 256 + qi * 128, 128),
                                       ts(nj, 512)], ost[:])
                return fn

            held_wo = []

            def enqueue_wo(qc, late=False, hold=0):
                for g in range(8):
                    item = (2048, qc, mk_wo_group(qc, g, late,
                                                  held=(g >= 8 - hold)))
                    if g >= 8 - hold:
                        held_wo.append(item)
                    else:
                        wo_q.append(item)

            def emit_wo_tail(qc):
                """Last chunk's W_o: per group, heads 0-2 accumulate first
                and the h3 matmul (gated on the final head's transpose) is
                deferred one PSUM-rotation behind, so PE has ~2.6us of
                ungated work while the last transposes land."""
                attnT = attnTs[qc]

                def open_group(g):
                    qi, nj = g // 4, g % 4
                    tag = "proj" if g in (4, 5) else "sc"
                    po = pps.tile([128, 512], f32, tag=tag,
                                  bufs=(2 if tag == "proj" else 4),
                                  name=f"wot{qc}{g}")
                    for hh in range(3):
                        nc.tensor.matmul(po[:],
                                         attnT[hh][:, ds(128 * qi, 128)],
                                         wo_sb[:, hh, ts(nj, 512)],
                                         start=(hh == 0), stop=False)
                    return po

                def close_group(g, po):
                    qi, nj = g // 4, g % 4
                    nc.tensor.matmul(po[:], attnT[3][:, ds(128 * qi, 128)],
                                     wo_sb[:, 3, ts(nj, 512)],
                                     start=False, stop=True)
                    ost = tail_stage()
                    if g % 2 == 0:
                        nc.scalar.activation(ost[:], po[:], AF.Copy)
                    else:
                        nc.vector.tensor_copy(out=ost[:], in_=po[:])
                    dmae = nc.sync if g % 2 == 0 else nc.gpsimd
                    dmae.dma_start(out[ds(qc * 256 + qi * 128, 128),
                                       ts(nj, 512)], ost[:])

                opened = []
                for g in range(8):
                    opened.append((g, open_group(g)))
                    if len(opened) == 6:
                        close_group(*opened.pop(0))
                while opened:
                    close_group(*opened.pop(0))

            # ---------------- attention scores + exp --------------------
            def emit_scores(h, qc, ki):
                qoff = 256 * (qc % 2)
                q_sb = qtiles[qc // 2]
                c0 = 128 if ki == 2 * qc + 1 else 0
                s12 = pps.tile([128, 512], f32, tag="sc", bufs=4,
                               name=f"s12_{h}_{qc}_{ki}")
                for m in (0, 1):
                    nc.tensor.matmul(
                        s12[:, ds(256 * m + c0, 256 - c0)],
                        k_sb[:, m, h, ts(ki, 128)],
                        q_sb[:, m, h, ds(qoff + c0, 256 - c0)],
                        start=(m == 0), stop=True, skip_group_check=True)
                s12v = s12[:].rearrange("p (m c) -> p m c", m=2)
                if ki == 2 * qc:        # diagonal block of subtile 0
                    nc.vector.tensor_tensor(s12v[:, :, 0:128],
                                            s12v[:, :, 0:128], dgv, OP.add)
                elif ki == 2 * qc + 1:  # diagonal block of subtile 1
                    nc.vector.tensor_tensor(s12v[:, :, 128:256],
                                            s12v[:, :, 128:256], dgv, OP.add)
                e12 = pe_pool.tile([128, 512], bf16, tag="e",
                                   name=f"e12_{h}_{qc}_{ki}")
                e12v = e12[:].rearrange("p (m c) -> p m c", m=2)
                nc.scalar.activation(e12v[:, :, c0:256], s12v[:, :, c0:256],
                                     AF.Exp, scale=SCALE)
                return e12

            # ---------------- per-(head, chunk) epilogue ----------------
            def epilogue(h, qc, U, sA, sS, attp):
                for j in (0, 1):
                    col = 2 * h + j
                    r1 = pep.tile([128, 1], f32, tag="r1")
                    r2 = pep.tile([128, 1], f32, tag="r2")
                    nc.vector.reciprocal(out=r1[:], in_=U[j][:, DH:DH + 1])
                    nc.vector.reciprocal(out=r2[:], in_=U[j][:, 259:260])
                    t2s = pep.tile([128, 128], f32, tag="t2s")
                    nc.vector.tensor_scalar(out=t2s[:], in0=U[j][:, 130:258],
                                            scalar1=r2[:], scalar2=None,
                                            op0=OP.mult)
                    ap = attp[col]
                    nc.vector.scalar_tensor_tensor(
                        out=ap[:], in0=U[j][:, 0:DH], scalar=r1[:],
                        in1=t2s[:], op0=OP.mult, op1=OP.subtract,
                        accum_out=sA[:, col:col + 1])
                    sqs = pep.tile([128, 128], bf16, tag="sqs")
                    nc.vector.scalar_tensor_tensor(
                        out=sqs[:], in0=ap[:], scalar=1.0, in1=ap[:],
                        op0=OP.mult, op1=OP.mult,
                        accum_out=sS[:, col:col + 1])

            # ---------------- LN finalize + transpose for one chunk -----
            def emit_norm(qc, sA, sS, attp, attnT):
                nmu = pst.tile([128, 8], f32, tag="nmu")
                ex2 = pst.tile([128, 8], f32, tag="ex2")
                nc.vector.tensor_scalar(out=nmu[:], in0=sA[:], scalar1=-1.0 / DH,
                                        scalar2=None, op0=OP.mult)
                nc.vector.tensor_scalar(out=ex2[:], in0=sS[:], scalar1=1.0 / DH,
                                        scalar2=None, op0=OP.mult)
                msq = pst.tile([128, 8], f32, tag="msq")
                nc.vector.tensor_tensor(msq[:], nmu[:], nmu[:], OP.mult)
                var = pst.tile([128, 8], f32, tag="var")
                nc.vector.tensor_tensor(var[:], ex2[:], msq[:], OP.subtract)
                # rsqrt(var + eps) via Quake bit-trick + 2 Newton steps
                # (keeps the whole LN finalize off ACT: no act-table thrash)
                vps = pst.tile([128, 8], f32, tag="vps")
                nc.vector.tensor_scalar(out=vps[:], in0=var[:], scalar1=1e-5,
                                        scalar2=None, op0=OP.add)
                i32 = mybir.dt.int32
                ysh = pst.tile([128, 8], f32, tag="ysh")
                nc.vector.tensor_scalar(out=ysh[:].bitcast(i32),
                                        in0=vps[:].bitcast(i32), scalar1=1,
                                        scalar2=None, op0=OP.arith_shift_right)
                y0 = pst.tile([128, 8], f32, tag="y0")
                nc.vector.tensor_scalar(out=y0[:].bitcast(i32),
                                        in0=ysh[:].bitcast(i32), scalar1=-1,
                                        scalar2=0x5f3759df, op0=OP.mult,
                                        op1=OP.add)
                rstd = y0
                for it in range(2):
                    yy = pst.tile([128, 8], f32, tag=f"yy{it}")
                    nc.vector.tensor_tensor(yy[:], rstd[:], rstd[:], OP.mult)
                    yv = pst.tile([128, 8], f32, tag=f"yv{it}")
                    nc.vector.tensor_tensor(yv[:], yy[:], vps[:], OP.mult)
                    yc = pst.tile([128, 8], f32, tag=f"yc{it}")
                    nc.vector.tensor_scalar(out=yc[:], in0=yv[:], scalar1=-0.5,
                                            scalar2=1.5, op0=OP.mult, op1=OP.add)
                    yn = pst.tile([128, 8], f32, tag=f"yn{it}")
                    nc.vector.tensor_tensor(yn[:], rstd[:], yc[:], OP.mult)
                    rstd = yn
                nbias = pst.tile([128, 8], f32, tag="nbias")
                nc.vector.tensor_tensor(nbias[:], nmu[:], rstd[:], OP.mult)
                for j in (0, 1):
                    for h in range(HPC):
                        col = 2 * h + j
                        attn = pat.tile([128, 128], bf16, tag="attn", bufs=4,
                                        name=f"attn{qc}{h}{j}")
                        nc.gpsimd.tensor_scalar(out=attn[:], in0=attp[col][:],
                                                scalar1=rstd[:, col:col + 1],
                                                scalar2=nbias[:, col:col + 1],
                                                op0=OP.mult, op1=OP.add)
                        nc.sync.dma_start_transpose(
                            attnT[h][:, ds(128 * j, 128)], attn[:])

            # per-head variant used for the last chunk: each head's
            # transposes land right after its epilogue instead of bunching
            # at the very end where nothing is left to hide their latency
            def emit_norm_head(qc, h, sA, sS, attp, attnT, pe_t=False):
                c0 = 2 * h
                nmu = pst.tile([128, 2], f32, tag="hnmu")
                ex2 = pst.tile([128, 2], f32, tag="hex2")
                nc.vector.tensor_scalar(out=nmu[:], in0=sA[:, c0:c0 + 2],
                                        scalar1=-1.0 / DH, scalar2=None,
                                        op0=OP.mult)
                nc.vector.tensor_scalar(out=ex2[:], in0=sS[:, c0:c0 + 2],
                                        scalar1=1.0 / DH, scalar2=None,
                                        op0=OP.mult)
                msq = pst.tile([128, 2], f32, tag="hmsq")
                nc.vector.tensor_tensor(msq[:], nmu[:], nmu[:], OP.mult)
                var = pst.tile([128, 2], f32, tag="hvar")
                nc.vector.tensor_tensor(var[:], ex2[:], msq[:], OP.subtract)
                vps = pst.tile([128, 2], f32, tag="hvps")
                nc.vector.tensor_scalar(out=vps[:], in0=var[:], scalar1=1e-5,
                                        scalar2=None, op0=OP.add)
                i32 = mybir.dt.int32
                ysh = pst.tile([128, 2], f32, tag="hysh")
                nc.vector.tensor_scalar(out=ysh[:].bitcast(i32),
                                        in0=vps[:].bitcast(i32), scalar1=1,
                                        scalar2=None, op0=OP.arith_shift_right)
                y0 = pst.tile([128, 2], f32, tag="hy0")
                nc.vector.tensor_scalar(out=y0[:].bitcast(i32),
                                        in0=ysh[:].bitcast(i32), scalar1=-1,
                                        scalar2=0x5f3759df, op0=OP.mult,
                                        op1=OP.add)
                rstd = y0
                for it in range(2):
                    yy = pst.tile([128, 2], f32, tag=f"hyy{it}")
                    nc.vector.tensor_tensor(yy[:], rstd[:], rstd[:], OP.mult)
                    yv = pst.tile([128, 2], f32, tag=f"hyv{it}")
                    nc.vector.tensor_tensor(yv[:], yy[:], vps[:], OP.mult)
                    yc = pst.tile([128, 2], f32, tag=f"hyc{it}")
                    nc.vector.tensor_scalar(out=yc[:], in0=yv[:], scalar1=-0.5,
                                            scalar2=1.5, op0=OP.mult, op1=OP.add)
                    yn = pst.tile([128, 2], f32, tag=f"hyn{it}")
                    nc.vector.tensor_tensor(yn[:], rstd[:], yc[:], OP.mult)
                    rstd = yn
                nbias = pst.tile([128, 2], f32, tag="hnbias")
                nc.vector.tensor_tensor(nbias[:], nmu[:], rstd[:], OP.mult)
                for j in (0, 1):
                    attn = pat.tile([128, 128], bf16, tag="attn", bufs=4,
                                    name=f"attn{qc}{h}{j}")
                    nc.vector.tensor_scalar(out=attn[:], in0=attp[c0 + j][:],
                                            scalar1=rstd[:, j:j + 1],
                                            scalar2=nbias[:, j:j + 1],
                                            op0=OP.mult, op1=OP.add)
                    if pe_t:
                        # last head of the last chunk: PE-transpose (~0.6us)
                        # instead of a ~3us DMA-transpose round trip -- PE is
                        # about to stall on exactly this data
                        pt = pps.tile([128, 128], bf16, tag="u1", bufs=1,
                                      name=f"ptT{qc}{h}{j}")
                        nc.tensor.transpose(pt[:], attn[:], id_sb[:])
                        nc.vector.tensor_copy(
                            out=attnT[h][:, ds(128 * j, 128)], in_=pt[:])
                    else:
                        # mid-kernel transposes stay off ACT: a blocked
                        # dispatch there stalls the exp FIFO behind it
                        nc.sync.dma_start_transpose(
                            attnT[h][:, ds(128 * j, 128)], attn[:])

            # ---------------- main fused loop ---------------------------
            units = [(h, qc, ki)
                     for qc in range(NQC)
                     for h in range(HPC)
                     for ki in range(2 * qc + 2)]
            pend = {}
            sp_ = [0]

            def ensure_scores(upto):
                while sp_[0] < len(units) and sp_[0] <= upto:
                    u = units[sp_[0]]
                    # q of chunk qc//2 and k/v of chunk ki//4 must be emitted
                    drain_proj((u[1], u[2]))
                    pend[sp_[0]] = emit_scores(*u)
                    sp_[0] += 1

            emit_proj0()
            enqueue_proj(1)
            uidx = 0
            for qc in range(NQC):
                last = qc == NQC - 1
                if qc >= 1:
                    # on the last chunk, hold a few W_o(qc-1) groups back as
                    # tail filler for the per-head transpose latency
                    enqueue_wo(qc - 1, hold=5 if last else 0)
                if qc % 2 == 0 and qc > 0 and qc // 2 + 1 < NCH:
                    enqueue_proj(qc // 2 + 1)
                sA = pst.tile([128, 8], f32, tag="sA", name=f"sA{qc}")
                sS = pst.tile([128, 8], f32, tag="sS", name=f"sS{qc}")
                attp = [pat.tile([128, 128], f32, tag=f"attp{i}",
                                 name=f"attp{qc}_{i}", bufs=1)
                        for i in range(8)]
                attnT = [pao.tile([128, 256], bf16, name=f"attnT{qc}_{hh}",
                                  tag=f"attnT{hh}") for hh in range(HPC)]
                attnTs[qc] = attnT
                for h in range(HPC):
                    U = [pps.tile([128, 260], f32, tag=f"u{j}", bufs=1,
                                  name=f"U{qc}_{h}_{j}") for j in (0, 1)]
                    for ki in range(2 * qc + 2):
                        ensure_scores(uidx + DEPTH)
                        e12 = pend.pop(uidx)
                        for j in (0, 1):
                            lim = 2 * qc + j
                            if ki > lim:
                                continue
                            for m in (0, 1):
                                nc.tensor.matmul(
                                    U[j][:, ds(130 * m, DH + 1 + m)],
                                    e12[:, ds(256 * m + 128 * j, 128)],
                                    v_sb[:, ki, h, 0:DH + 1 + m],
                                    start=(ki == 0 and m == 0),
                                    stop=(ki == lim and m == 1),
                                    skip_group_check=True)
                        pump(FILL)
                        uidx += 1
                    epilogue(h, qc, U, sA, sS, attp)
                    if last and h < HPC - 1:
                        emit_norm_head(qc, h, sA, sS, attp, attnT)
                if not last:
                    emit_norm(qc, sA, sS, attp, attnT)
                else:
                    # run the held W_o groups now: they keep PE busy while
                    # the last head's norm chain runs on DVE, and its
                    # PE-transpose then lands with almost no stall
                    for _c, _k, fn in held_wo:
                        fn()
                    held_wo.clear()
                    emit_norm_head(qc, HPC - 1, sA, sS, attp, attnT,
                                   pe_t=True)
                # attnT pool is 3 deep: W_o of qc-2 must fully drain here
                drain_wo(qc - 2)

            # ---------------- tail: last chunk's W_o --------------------
            drain_proj((NQC, 99))
            drain_wo(NQC)              # leftover W_o of qc-1
            emit_wo_tail(NQC - 1)

    nc.finalize()
    return nc


def get_program(s=S):
    if s not in _PROGRAM_CACHE:
        _PROGRAM_CACHE[s] = build_program(s)
    return _PROGRAM_CACHE[s]


def make_core_inputs(x, cos, sin, W_qkv, W_o, ln_gamma, lbda, core, s=S):
    """Host-side shard prep for one core."""
    b, hg = core // 4, core % 4
    heads = list(range(hg * HPC, (hg + 1) * HPC))

    def qk_block_cols(base, dstart):
        # even/odd pair columns for one 32-wide block across the 4 heads
        return [base + hh * DH + dstart + 2 * p for hh in heads for p in range(32)]

    cols = []
    for base in (0, D):                       # q section, k section
        for dstart in (0, 1, HD, HD + 1):     # m1-even, m1-odd, m2-even, m2-odd
            cols += qk_block_cols(base, dstart)
    wqk = np.ascontiguousarray(W_qkv[:, cols]).astype(bf16_np)
    vcols = [2 * D + hh * DH + dd for hh in heads for dd in range(DH)]
    wv = np.ascontiguousarray(W_qkv[:, vcols]).astype(bf16_np)
    worows = [hh * DH + dd for hh in heads for dd in range(DH)]
    gamma_scale = np.concatenate([ln_gamma[hh] * OUT_MULT for hh in heads])
    wo = np.ascontiguousarray(W_o[worows, :] * gamma_scale[:, None]).astype(bf16_np)

    xT = np.ascontiguousarray(x[b].T).astype(bf16_np)
    cst = np.ascontiguousarray(np.tile(cos.T, (HPC, 1))).astype(bf16_np)
    snt = np.ascontiguousarray(np.tile(sin.T, (HPC, 1))).astype(bf16_np)

    diag = np.where(np.triu(np.ones((128, 128), dtype=bool)), 0.0, NEG)
    dg2 = np.concatenate([diag, diag], axis=1).astype(np.float32)

    lam2 = np.zeros((128, 2), dtype=np.float32)
    lam2[:, 0] = 1.0
    lam2[:, 1] = 1.0 / max(float(lbda), 1e-6)

    return {
        "xT": xT, "wqk": wqk, "wv": wv, "wo": wo, "cs": cst, "sn": snt,
        "dg": dg2, "lamv": lam2.astype(bf16_np),
        "ident": np.eye(128, dtype=bf16_np),
    }


def _mask_is_causal(mask, s=S):
    m = np.asarray(mask).reshape(s, s)
    tril = np.tril(np.ones((s, s), dtype=bool))
    if not np.array_equal(m == 0.0, tril):
        return False
    off = m[~tril]
    return off.size == 0 or (np.all(off <= -1.0e8) and np.all(np.isfinite(off)))


def _numpy_reference(x, mask, cos, sin, W_qkv, W_o, ln_gamma, ln_beta, lbda):
    """Exact-math fallback (used only if the mask is not the causal pattern)."""
    b, s, d = x.shape
    qkv = x @ W_qkv
    q, k, v = np.split(qkv, 3, axis=-1)
    q = q.reshape(b, s, H, DH).transpose(0, 2, 1, 3)
    k = k.reshape(b, s, H, DH).transpose(0, 2, 1, 3)
    v = v.reshape(b, s, H, DH).transpose(0, 2, 1, 3)

    def rope(t):
        tr = t.reshape(b, H, s, HD // 2, 2)
        x1, x2 = tr[..., 0], tr[..., 1]
        c = cos[None, None]
        sn_ = sin[None, None]
        o1 = x1 * c - x2 * sn_
        o2 = x1 * sn_ + x2 * c
        return np.stack([o1, o2], axis=-1).reshape(b, H, s, HD)

    q1, q2 = q[..., :HD], q[..., HD:]
    k1, k2 = k[..., :HD], k[..., HD:]
    q1, k1 = rope(q1), rope(k1)
    q2, k2 = rope(q2), rope(k2)

    def softm(z):
        z = z - z.max(-1, keepdims=True)
        e = np.exp(z)
        return e / e.sum(-1, keepdims=True)

    m = np.asarray(mask).reshape(1, 1, s, s)
    a1 = softm(np.einsum("bhqd,bhkd->bhqk", q1, k1) * SCALE + m)
    a2 = softm(np.einsum("bhqd,bhkd->bhqk", q2, k2) * SCALE + m)
    a = a1 - float(lbda) * a2
    o = np.einsum("bhqk,bhkd->bhqd", a, v)
    mu = o.mean(-1, keepdims=True)
    var = o.var(-1, keepdims=True)
    o = (o - mu) / np.sqrt(var + 1e-5)
    o = o * ln_gamma[None, :, None, :] + ln_beta[None, :, None, :]
    o = o * OUT_MULT
    o = o.transpose(0, 2, 1, 3).reshape(b, s, d)
    return (o @ W_o).astype(np.float32)


def kernel(x, mask, cos, sin, W_qkv, W_o, ln_gamma, ln_beta, lbda):
    global LAST_RESULTS, LAST_EXEC_NS
    x = np.asarray(x, dtype=np.float32)
    cos = np.asarray(cos, dtype=np.float32)
    sin = np.asarray(sin, dtype=np.float32)
    W_qkv = np.asarray(W_qkv, dtype=np.float32)
    W_o = np.asarray(W_o, dtype=np.float32)
    ln_gamma = np.asarray(ln_gamma, dtype=np.float32)
    ln_beta = np.asarray(ln_beta, dtype=np.float32)
    lbda_f = float(np.asarray(lbda))

    if not _mask_is_causal(mask):
        return _numpy_reference(x, mask, cos, sin, W_qkv, W_o,
                                ln_gamma, ln_beta, lbda_f)

    from concourse.bass_utils import run_bass_kernel_spmd

    nc = get_program(S)
    in_maps = [
        make_core_inputs(x, cos, sin, W_qkv, W_o, ln_gamma, lbda_f, c)
        for c in range(NCORES)
    ]
    kwargs = {"trace": TRACE}
    if TRACE and TRACE_DIR:
        kwargs["tmpdir"] = TRACE_DIR
    res = run_bass_kernel_spmd(nc, in_maps, core_ids=list(range(NCORES)),
                               **kwargs)
    LAST_RESULTS = res
    LAST_EXEC_NS = getattr(res, "exec_time_ns", None)

    outf = np.zeros((B, S, D), dtype=np.float32)
    for c in range(NCORES):
        outf[c // 4] += res.results[c]["out"].astype(np.float32)
    # ln_beta rank-1 term: (beta * OUT_MULT) @ W_o added to every token
    beta_term = (ln_beta.reshape(-1) * OUT_MULT) @ W_o
    outf += beta_term[None, None, :]
    return outf
